# revision 1
# baseline (speedup 1.0000x reference)
"""BKT (Bayesian Knowledge Tracing) forward pass on Trainium2, 8 NeuronCores.

The reference's chunked 32-trajectory scan is mathematically a 2-state HMM
forward pass. Per (sequence, t):
    alpha' = alpha @ (diag(o_t) @ Tr)      (row vector times matrix)
with o_s(t) = P(obs_t | L=s), Tr the 2x2 BKT transition matrix, and
    out_c(t) = log(alpha@pc) - log(alpha@1),  pc = [P(c|0), P(c|1)].

Device algorithm (per core, batch-parallel over 2048 sequences laid out as
128 partitions x 16 groups, free dim = (t, g)):
  1. ACT sigmoids give observation probs; the corr-select is folded into the
     sigmoid argument via sign flip: o0 = sigmoid((2c-1)*lg).
  2. Per-step 2x2 matrices W_t, chunk products A_c over K=10 steps built with
     fused stride-0-broadcast tensor_tensor folds (parallel over chunks).
  3. Short serial recursion over chunk matrices -> chunk-start alphas.
  4. Within-chunk recovery (parallel over chunks) -> per-t alphas.
  5. Predictions + log-softmax via Ln(num*s) - Ln(den*s), s = 2^-exp(den)
     (exact power-of-two rescale keeps the ACT Ln LUT in its sane range).

Sharding: pure data-parallel over batch; parameter tables are gathered on
host (traffic-neutral: 8B/element of gathered logits replaces the 8B int64
problem id), all recurrences stay on-device.
"""

import numpy as np

import concourse.bass as bass
import concourse.bacc as bacc
import concourse.tile as tile
import concourse.mybir as mybir
from concourse._compat import with_exitstack

F32 = mybir.dt.float32
U8 = mybir.dt.uint8
AF = mybir.ActivationFunctionType
OP = mybir.AluOpType

P = 128          # partitions
N_CORES = 8


def emit_bkt(nc, G, T, K, SEG, renorm_every=2):
    """Emit the BKT kernel for one core. Sequences = P*G, free layout (t, g).

    Software-pipelined over T-segments: segment s+1's observation sigmoids
    (ACT) are emitted before segment s's Ln calls, and segment s's final
    log-subtract is emitted after segment s+1's W-build, so neither engine
    stalls on the other at segment boundaries.

    DRAM tensors:
      lls:  (P, T, 2, G) f32  packed [guess, slip] logits
      cm:   (P, T, G) i8      2*corr-1 in {-1, +1}
      dyn:  (P, 3, G) f32     [logit_pL, logit_pF, logit_pI0]
      out:  (P, T, 2, G) f32  [log p(incorrect), log p(correct)]
    """
    assert T % SEG == 0 and SEG % K == 0
    NSEG = T // SEG
    CS = SEG // K          # chunks per segment
    CT = T // K            # total chunks

    lls_d = nc.dram_tensor("lls", [P, T, 2, G], F32, kind="ExternalInput")
    cm_d = nc.dram_tensor("cm", [P, T, G], mybir.dt.int8, kind="ExternalInput")
    dyn_d = nc.dram_tensor("dyn", [P, 3, G], F32, kind="ExternalInput")
    out_d = nc.dram_tensor("out", [P, T, 2, G], F32, kind="ExternalOutput")

    with tile.TileContext(nc) as tc:
        with (
            tc.tile_pool(name="singles", bufs=1) as singles,
            tc.tile_pool(name="io", bufs=2) as io,
            tc.tile_pool(name="work", bufs=1) as work,
            tc.tile_pool(name="actb", bufs=2) as actb,
        ):
            # ---- per-sequence constants ----
            dyn_t = singles.tile([P, 3, G], F32)
            nc.sync.dma_start(dyn_t[:], dyn_d[:])
            # Tr packed [s][s']: [[1-l, l], [f, 1-f]]; 1-sigmoid(x) = sigmoid(-x)
            Tp = singles.tile([P, 2, G, 2], F32)   # [s][g][s']
            nc.scalar.activation(Tp[:, 0, :, 0], dyn_t[:, 0, :], AF.Sigmoid, scale=-1.0)
            nc.scalar.activation(Tp[:, 0, :, 1], dyn_t[:, 0, :], AF.Sigmoid)
            nc.scalar.activation(Tp[:, 1, :, 0], dyn_t[:, 1, :], AF.Sigmoid)
            nc.scalar.activation(Tp[:, 1, :, 1], dyn_t[:, 1, :], AF.Sigmoid, scale=-1.0)

            # chunk-start alphas, all chunks + final carry
            starts = singles.tile([P, CT + 1, 2, G], F32)
            nc.scalar.activation(starts[:, 0, 0, :], dyn_t[:, 2, :], AF.Sigmoid, scale=-1.0)
            nc.scalar.activation(starts[:, 0, 1, :], dyn_t[:, 2, :], AF.Sigmoid)

            obs = {}        # per-seg live tiles from phase A
            fin = {}        # per-seg live tiles awaiting finalize

            def phase_a(seg, nsplit=1):
                """Loads + observation sigmoids for segment seg. nsplit > 1
                slices the DMA + sigmoid chain so compute starts on the first
                slice while later slices are still in flight (startup ramp)."""
                s0 = seg * SEG
                lls = io.tile([P, SEG, 2, G], F32, tag="lls")
                cmt = io.tile([P, SEG, G], mybir.dt.int8, tag="cm")
                zpk = work.tile([P, SEG, 2, G], F32, tag="zpk")
                op_t = actb.tile([P, SEG, 2, G], F32, tag="opack")
                ptp = actb.tile([P, SEG, 2, G], F32, tag="ptp")
                bounds = [SEG * h // nsplit for h in range(nsplit + 1)]
                for h in range(nsplit):
                    a, b = bounds[h], bounds[h + 1]
                    nc.sync.dma_start(lls[:, a:b], lls_d[:, s0 + a : s0 + b, :, :])
                    nc.sync.dma_start(cmt[:, a:b], cm_d[:, s0 + a : s0 + b, :])
                    # o_s(t) = sigmoid(+-logit): corr-select via sign flip
                    nc.vector.tensor_tensor(
                        zpk[:, a:b], lls[:, a:b],
                        cmt[:, a:b].unsqueeze(2).broadcast_to((P, b - a, 2, G)),
                        OP.mult,
                    )
                    nc.scalar.activation(op_t[:, a:b, 0, :], zpk[:, a:b, 0, :], AF.Sigmoid)
                    nc.scalar.activation(op_t[:, a:b, 1, :], zpk[:, a:b, 1, :], AF.Sigmoid, scale=-1.0)
                    # true-outcome probs for predictions: [P(c|0), P(c|1)]
                    nc.scalar.activation(ptp[:, a:b, 0, :], lls[:, a:b, 0, :], AF.Sigmoid)
                    nc.scalar.activation(ptp[:, a:b, 1, :], lls[:, a:b, 1, :], AF.Sigmoid, scale=-1.0)
                obs[seg] = (op_t, ptp)

            def finalize(seg):
                """Log-subtract + store for segment seg (after its ACT Lns)."""
                s0 = seg * SEG
                out_t, _ = fin.pop(seg)
                h = SEG // 2
                nc.sync.dma_start(out_d[:, s0 : s0 + h, :, :], out_t[:, :h])
                nc.sync.dma_start(out_d[:, s0 + h : s0 + SEG, :, :], out_t[:, h:])

            def phase_b(seg):
                """W-build, folds, serial recursion, recovery, predictions."""
                c0 = seg * CS
                op_t, ptp = obs.pop(seg)

                # per-step matrices W[t][s][s'][g] = o_s(t) * Tr[s][s']
                Wp = work.tile([P, SEG, 2, G, 2], F32, tag="Wp")   # [t][s][g][s']
                nc.vector.tensor_tensor(
                    Wp[:],
                    op_t[:].unsqueeze(4).broadcast_to((P, SEG, 2, G, 2)),
                    Tp[:].unsqueeze(1).broadcast_to((P, SEG, 2, G, 2)),
                    OP.mult,
                )
                Wc = Wp[:].rearrange("p (c k) s g t -> p c k s g t", k=K)

                if seg >= 1:
                    finalize(seg - 1)

                # chunk products A_c = W_{ck} @ ... @ W_{ck+K-1}
                A = work.tile([P, CS, 2, 2, G], F32, tag="A")      # [c][i][s'][g]
                Ax = A[:].rearrange("p c i s g -> p c i g s")      # iterate (c,i,g,s')
                TM = work.tile([P, CS, 2, 2, G, 2], F32, tag="TM")  # [c][i][m][g][s']
                nc.scalar.copy(Ax, Wc[:, :, 0])
                for j in range(1, K):
                    Wj = Wc[:, :, j]      # (P, CS, 2, G, 2) = [c][m][g][s']
                    # TM[i,m,g,s'] = A[i,m]*W[m,s'] in one op (APs merge <=3D),
                    # then A'[i,s'] = TM[i,0,s'] + TM[i,1,s']
                    nc.vector.tensor_tensor(
                        TM[:],
                        A[:].unsqueeze(5).broadcast_to((P, CS, 2, 2, G, 2)),
                        Wj[:].unsqueeze(2).broadcast_to((P, CS, 2, 2, G, 2)),
                        OP.mult,
                    )
                    nc.vector.tensor_tensor(Ax, TM[:, :, :, 0], TM[:, :, :, 1], OP.add)

                # serial chunk recursion:
                # sv[m][s'] = starts[m]*A[m,s'] ; starts' = sv[0]+sv[1]
                sv = work.tile([P, 2, 2, G], F32, tag="sv")
                ssum = work.tile([P, G], F32, tag="ssum")
                for cl in range(CS):
                    cg = c0 + cl
                    st = starts[:, cg]
                    stn = starts[:, cg + 1]
                    nc.vector.tensor_tensor(
                        sv[:],
                        st[:].unsqueeze(2).broadcast_to((P, 2, 2, G)),
                        A[:, cl],
                        OP.mult,
                    )
                    nc.vector.tensor_tensor(stn, sv[:, 0], sv[:, 1], OP.add)
                    if cg % renorm_every == renorm_every - 1:
                        nc.vector.tensor_tensor(
                            ssum[:], stn[:, 0, :], stn[:, 1, :], OP.add
                        )
                        nc.vector.reciprocal_approx_fast(ssum[:], ssum[:])
                        nc.vector.tensor_tensor(
                            stn,
                            stn,
                            ssum[:].unsqueeze(1).broadcast_to((P, 2, G)),
                            OP.mult,
                        )

                # within-chunk recovery: per-t alphas
                rec = work.tile([P, SEG, 2, G], F32, tag="rec")
                rc = rec[:].rearrange("p (c k) s g -> p c k s g", k=K)
                nc.scalar.copy(rc[:, :, 0], starts[:, c0 : c0 + CS])
                RR = work.tile([P, CS, 2, G, 2], F32, tag="RR")   # [c][m][g][s']
                for j in range(1, K):
                    prev = rc[:, :, j - 1]   # (P, CS, 2, G) = [c][m][g]
                    nc.vector.tensor_tensor(
                        RR[:],
                        prev[:].unsqueeze(4).broadcast_to((P, CS, 2, G, 2)),
                        Wc[:, :, j - 1],
                        OP.mult,
                    )
                    nc.vector.tensor_tensor(
                        rc[:, :, j].rearrange("p c s g -> p c g s"),
                        RR[:, :, 0], RR[:, :, 1], OP.add,
                    )

                # predictions; the last segment runs in halves so its Ln +
                # store overlap the second half's vector work (tail exposure)
                qp = work.tile([P, SEG, 2, G], F32, tag="qp")
                pn = work.tile([P, SEG, 2, G], F32, tag="pn")
                den = work.tile([P, SEG, G], F32, tag="den")
                rr = work.tile([P, SEG, G], F32, tag="rr")
                out_t = io.tile([P, SEG, 2, G], F32, tag="out")
                nsp = 2 if seg == NSEG - 1 else 1
                bounds = [SEG * h // nsp for h in range(nsp + 1)]
                for hh in range(nsp):
                    a, b = bounds[hh], bounds[hh + 1]
                    n = b - a
                    nc.vector.tensor_tensor(qp[:, a:b], rec[:, a:b], ptp[:, a:b], OP.mult)
                    # pn[t][1] = num (correct mass), pn[t][0] = den - num
                    nc.vector.tensor_tensor(
                        pn[:, a:b, 1, :], qp[:, a:b, 0, :], qp[:, a:b, 1, :], OP.add
                    )
                    nc.vector.tensor_tensor(
                        den[:, a:b], rec[:, a:b, 0, :], rec[:, a:b, 1, :], OP.add
                    )
                    nc.vector.tensor_tensor(
                        pn[:, a:b, 0, :], den[:, a:b], pn[:, a:b, 1, :], OP.subtract
                    )
                    # Normalize by r ~= 1/den (~51 ULP): out = Ln(pn*r). The
                    # approximation error shifts both outputs by -Ln(den*r)
                    # ~ 4e-6 (harmless), avoids the Ln LUT's bad range below
                    # ~2^-50, and replaces the exponent-rescale pipeline.
                    nc.vector.reciprocal_approx_fast(rr[:, a:b], den[:, a:b])
                    nc.vector.tensor_tensor(
                        pn[:, a:b], pn[:, a:b],
                        rr[:, a:b].unsqueeze(2).broadcast_to((P, n, 2, G)), OP.mult,
                    )
                    m = (a + b) // 2
                    nc.scalar.activation(out_t[:, a:m], pn[:, a:m], AF.Ln)
                    nc.scalar.activation(out_t[:, m:b], pn[:, m:b], AF.Ln)
                fin[seg] = (out_t, None)

            for seg in range(NSEG):
                phase_a(seg, nsplit=(4 if seg == 0 else 1))
                if seg >= 1:
                    phase_b(seg - 1)
            phase_b(NSEG - 1)
            finalize(NSEG - 1)

    return nc


# ------------------------------------------------------------------
# Host-side full-problem wrapper
# ------------------------------------------------------------------

_B, _T, _K, _SEG = 16384, 500, 10, 100
_G = _B // (P * N_CORES)   # 16 groups per core

_cached = {}


def _build():
    if "nc" not in _cached:
        nc = bacc.Bacc(None, target_bir_lowering=False)
        emit_bkt(nc, G=_G, T=_T, K=_K, SEG=_SEG)
        nc.compile()
        _cached["nc"] = nc
    return _cached["nc"]


def _shard(arr, core):
    """(B,...) -> this core's (P, ..., G) permuted view, seq = g*128 + p."""
    rows = arr[core * P * _G : (core + 1) * P * _G]
    r = rows.reshape(_G, P, *arr.shape[1:])
    order = (1,) + tuple(range(2, r.ndim)) + (0,)
    return np.ascontiguousarray(r.transpose(order))


def kernel(corr, kc, problem, dynamics_logits_table, obs_logits_kc,
           obs_logits_problem, fastbkt_n):
    from concourse.bass_utils import run_bass_kernel_spmd

    corr = np.asarray(corr, dtype=np.float32)
    kc = np.asarray(kc).astype(np.int64)
    problem = np.asarray(problem).astype(np.int64)
    dyn_table = np.asarray(dynamics_logits_table, dtype=np.float32)
    obs_kc = np.asarray(obs_logits_kc, dtype=np.float32)
    obs_prob = np.asarray(obs_logits_problem, dtype=np.float32)

    B, T = corr.shape
    assert B == _B and T == _T, (B, T)

    # host gathers (traffic-neutral input marshaling)
    lls = obs_kc[kc][:, None, :] + obs_prob[problem]       # (B, T, 2)
    dyn = dyn_table[kc]                                    # (B, 3)
    cm8 = (corr * 2.0 - 1.0).astype(np.int8)

    nc = _build()
    in_maps = []
    for core in range(N_CORES):
        in_maps.append({
            "lls": _shard(lls, core),
            "cm": _shard(cm8, core),
            "dyn": _shard(dyn, core),
        })

    res = run_bass_kernel_spmd(
        nc, in_maps, core_ids=list(range(N_CORES)), **_cached.get("run_kwargs", {})
    )
    _cached["last_results"] = res

    out = np.empty((B, T, 2), np.float32)
    for core in range(N_CORES):
        o = res.results[core]["out"]                       # (P, T, 2, G)
        rows = o.transpose(3, 0, 1, 2).reshape(P * _G, T, 2)
        out[core * P * _G : (core + 1) * P * _G] = rows
    return out



# revision 8
# speedup vs baseline: 1.7495x; 1.7495x over previous
"""BKT (Bayesian Knowledge Tracing) forward pass on Trainium2, 8 NeuronCores.

The reference's chunked 32-trajectory scan is a 2-state HMM forward pass.
Per (sequence, t):  W_t = diag(o_t) @ Tr  (2x2 per-step matrix),
    alpha' = alpha @ W_t,   p_corr(t) = (alpha . pc_t) / (alpha . 1),
    out = [Ln(1-p), Ln(p)].
Alphas are kept UNNORMALIZED pairs throughout (scale cancels in the
prediction ratio); only occasional power-limited renorms keep fp range.

Device algorithm per core (2048 seqs = 128 partitions x 16 groups, layout
[p, t, s, s', g] with g innermost so every bf16 op hits the DVE 2x mode):
  1. Pool: zpk = lls2 * cm (sign-fold; host pre-negates the slip logit so a
     single sigmoid yields both obs probs).  ACT: op = sigmoid(zpk) -> bf16,
     pc = sigmoid(lls2) -> fp32.
  2. DVE bf16: W = op x Tr; chunk products A_c over K=10 steps (2 fused
     broadcast tensor_tensor per fold, parallel across chunks).
  3. Pool fp32: serial alpha-pair recursion over chunk matrices (2 ops per
     chunk); DVE renorm (approx-recip) every 2 chunks bounds the range.
  4. DVE bf16: within-chunk alpha recovery (1 mult + 1 add per step).
  5. fp32 predictions: qp = alpha*pc, num/den sums, approx-recip, q = 1-p
     (fp32 keeps the cancellation harmless), Ln on ACT, bf16 store (host
     upcasts, which halves the output traffic).

Sharding: pure data-parallel over batch; the tiny per-KC/per-problem tables
are gathered on host (traffic-neutral marshaling), recurrences on device.
"""

import numpy as np

import concourse.bass as bass
import concourse.bacc as bacc
import concourse.tile as tile
import concourse.mybir as mybir

F32 = mybir.dt.float32
BF16 = mybir.dt.bfloat16
I8 = mybir.dt.int8
AF = mybir.ActivationFunctionType
OP = mybir.AluOpType

P = 128
N_CORES = 8


def emit_bkt(nc, G, T, K, SEG, RN=2):
    assert T % SEG == 0 and SEG % K == 0
    NSEG = T // SEG
    CS = SEG // K
    CT = T // K

    lls_d = nc.dram_tensor("lls2", [P, T, 2, G], F32, kind="ExternalInput")
    cm_d = nc.dram_tensor("cm", [P, T, G], I8, kind="ExternalInput")
    dyn_d = nc.dram_tensor("dyn", [P, 3, G], F32, kind="ExternalInput")
    out_d = nc.dram_tensor("out", [P, T, 2, G], BF16, kind="ExternalOutput")

    with tile.TileContext(nc) as tc:
        with (
            tc.tile_pool(name="singles", bufs=1) as singles,
            tc.tile_pool(name="io", bufs=2) as io,
            tc.tile_pool(name="wk2", bufs=2) as wk2,
            tc.tile_pool(name="wk1", bufs=1) as wk1,
        ):
            # ---- per-sequence constants ----
            dyn_t = singles.tile([P, 3, G], F32)
            nc.sync.dma_start(dyn_t[:], dyn_d[:])
            # Tr[s][s'][g]: [[1-l, l], [f, 1-f]]
            Tp = singles.tile([P, 2, 2, G], BF16)
            nc.scalar.activation(Tp[:, 0, 0], dyn_t[:, 0, :], AF.Sigmoid, scale=-1.0)
            nc.scalar.activation(Tp[:, 0, 1], dyn_t[:, 0, :], AF.Sigmoid)
            nc.scalar.activation(Tp[:, 1, 0], dyn_t[:, 1, :], AF.Sigmoid)
            nc.scalar.activation(Tp[:, 1, 1], dyn_t[:, 1, :], AF.Sigmoid, scale=-1.0)
            # chunk-start alpha pairs (unnormalized); start = (1-pI0, pI0)
            starts = singles.tile([P, CT + 1, 2, G], F32)
            nc.scalar.activation(starts[:, 0, 0], dyn_t[:, 2, :], AF.Sigmoid, scale=-1.0)
            nc.scalar.activation(starts[:, 0, 1], dyn_t[:, 2, :], AF.Sigmoid)

            obs = {}
            fin = {}

            def phase_a(seg, nsplit=1):
                s0 = seg * SEG
                lls = io.tile([P, SEG, 2, G], F32, tag="lls")
                cmt = io.tile([P, SEG, G], I8, tag="cm")
                zpk = wk2.tile([P, SEG, 2, G], F32, tag="zpk")
                op_t = wk2.tile([P, SEG, 2, G], BF16, tag="op")
                pc_t = wk2.tile([P, SEG, 2, G], F32, tag="pc")
                bounds = [SEG * h // nsplit for h in range(nsplit + 1)]
                for h in range(nsplit):
                    a, b = bounds[h], bounds[h + 1]
                    nc.sync.dma_start(lls[:, a:b], lls_d[:, s0 + a : s0 + b])
                    nc.sync.dma_start(cmt[:, a:b], cm_d[:, s0 + a : s0 + b])
                    # zpk = lls2 * (2c-1): folds corr into both logits
                    nc.gpsimd.tensor_tensor(
                        zpk[:, a:b], lls[:, a:b],
                        cmt[:, a:b].unsqueeze(2).broadcast_to((P, b - a, 2, G)),
                        OP.mult,
                    )
                    # op = [P(obs|L=0), P(obs|L=1)]; pc = [P(c|0), P(c|1)]
                    nc.scalar.activation(op_t[:, a:b], zpk[:, a:b], AF.Sigmoid)
                    nc.scalar.activation(pc_t[:, a:b], lls[:, a:b], AF.Sigmoid)
                obs[seg] = (op_t, pc_t)

            def finalize(seg):
                s0 = seg * SEG
                out_t = fin.pop(seg)
                nc.sync.dma_start(out_d[:, s0 : s0 + SEG], out_t[:])

            def phase_b(seg):
                c0 = seg * CS
                op_t, pc_t = obs.pop(seg)

                # W[t][s][s'][g] = op_s(t) * Tr[s][s']  (g innermost: 2x mode)
                Wp = wk2.tile([P, SEG, 2, 2, G], BF16, tag="Wp")
                nc.vector.tensor_tensor(
                    Wp[:],
                    op_t[:].unsqueeze(3).broadcast_to((P, SEG, 2, 2, G)),
                    Tp[:].unsqueeze(1).broadcast_to((P, SEG, 2, 2, G)),
                    OP.mult,
                )
                Wc = Wp[:].rearrange("p (c k) s t g -> p c k s t g", k=K)

                if seg >= 1:
                    finalize(seg - 1)

                # chunk products A_c = W_c0 @ ... @ W_c,K-1  ([c, i, s', g])
                A = wk2.tile([P, CS, 2, 2, G], BF16, tag="A")
                TM = wk2.tile([P, CS, 2, 2, 2, G], BF16, tag="TM")
                nc.vector.tensor_scalar(A[:], Wc[:, :, 0], 1.0, 0.0, OP.mult, OP.add)
                for j in range(1, K):
                    nc.vector.tensor_tensor(
                        TM[:],
                        A[:].unsqueeze(4).broadcast_to((P, CS, 2, 2, 2, G)),
                        Wc[:, :, j].unsqueeze(2).broadcast_to((P, CS, 2, 2, 2, G)),
                        OP.mult,
                    )
                    nc.vector.tensor_tensor(A[:], TM[:, :, :, 0], TM[:, :, :, 1], OP.add)

                # serial alpha-pair recursion over chunks (Pool, fp32)
                sv = wk1.tile([P, 2, 2, G], F32, tag="sv")
                rcp = wk1.tile([P, G], F32, tag="rcp")
                for cl in range(CS):
                    cg = c0 + cl
                    nc.gpsimd.tensor_tensor(
                        sv[:],
                        starts[:, cg].unsqueeze(2).broadcast_to((P, 2, 2, G)),
                        A[:, cl], OP.mult,
                    )
                    nc.gpsimd.tensor_tensor(
                        starts[:, cg + 1], sv[:, 0], sv[:, 1], OP.add
                    )
                    if cg % RN == RN - 1:
                        nc.vector.reciprocal_approx_fast(
                            rcp[:], starts[:, cg + 1, 0]
                        )
                        nc.vector.tensor_tensor(
                            starts[:, cg + 1],
                            starts[:, cg + 1],
                            rcp[:].unsqueeze(1).broadcast_to((P, 2, G)),
                            OP.mult,
                        )

                # within-chunk alpha recovery (bf16): a_j = a_{j-1} @ W_{j-1}
                rec = wk2.tile([P, CS, K, 2, G], BF16, tag="rec")
                nc.gpsimd.tensor_scalar(
                    rec[:, :, 0], starts[:, c0 : c0 + CS], 1.0, 0.0, OP.mult, OP.add
                )
                # RR layout [c, s', i, g]; one mult per target state s'
                RR = wk1.tile([P, CS, 2, 2, G], BF16, tag="RR")
                for j in range(1, K):
                    for sp in range(2):
                        nc.vector.tensor_tensor(
                            RR[:, :, sp],
                            rec[:, :, j - 1],
                            Wc[:, :, j - 1, :, sp, :], OP.mult,
                        )
                    nc.vector.tensor_tensor(
                        rec[:, :, j], RR[:, :, :, 0], RR[:, :, :, 1], OP.add
                    )

                # predictions (fp32): p = (a.pc)/(a.1), q = 1-p
                af = rec[:].rearrange("p c k s g -> p (c k) s g")
                qp0 = wk1.tile([P, SEG, G], F32, tag="qp0")
                qp1 = wk1.tile([P, SEG, G], F32, tag="qp1")
                num = wk1.tile([P, SEG, G], F32, tag="num")
                den = wk1.tile([P, SEG, G], F32, tag="den")
                rdn = wk1.tile([P, SEG, G], F32, tag="rdn")
                pt = wk1.tile([P, SEG, G], F32, tag="pt")
                qt = wk1.tile([P, SEG, G], F32, tag="qt")
                out_t = io.tile([P, SEG, 2, G], BF16, tag="out")
                nc.gpsimd.tensor_tensor(qp0[:], af[:, :, 0], pc_t[:, :, 0], OP.mult)
                nc.vector.tensor_tensor(qp1[:], af[:, :, 1], pc_t[:, :, 1], OP.mult)
                nc.vector.tensor_tensor(num[:], qp0[:], qp1[:], OP.add)
                nc.vector.tensor_tensor(den[:], af[:, :, 0], af[:, :, 1], OP.add)
                nc.vector.reciprocal_approx_fast(rdn[:], den[:])
                nc.vector.tensor_tensor(pt[:], num[:], rdn[:], OP.mult)
                nc.vector.tensor_scalar(qt[:], pt[:], -1.0, 1.0, OP.mult, OP.add)
                nc.scalar.activation(out_t[:, :, 0], qt[:], AF.Ln)
                nc.scalar.activation(out_t[:, :, 1], pt[:], AF.Ln)
                fin[seg] = out_t

            for seg in range(NSEG):
                phase_a(seg, nsplit=(4 if seg == 0 else 1))
                if seg >= 1:
                    phase_b(seg - 1)
            phase_b(NSEG - 1)
            finalize(NSEG - 1)

    return nc


# ------------------------------------------------------------------
# Host-side full-problem wrapper
# ------------------------------------------------------------------

_B, _T, _K, _SEG = 16384, 500, 10, 100
_G = _B // (P * N_CORES)

_cached = {}


def _build():
    if "nc" not in _cached:
        nc = bacc.Bacc(None, target_bir_lowering=False)
        emit_bkt(nc, G=_G, T=_T, K=_K, SEG=_SEG)
        nc.compile()
        _cached["nc"] = nc
    return _cached["nc"]


def _shard(arr, core):
    """(B,...) -> this core's (P, ..., G) permuted view, seq = g*128 + p."""
    rows = arr[core * P * _G : (core + 1) * P * _G]
    r = rows.reshape(_G, P, *arr.shape[1:])
    order = (1,) + tuple(range(2, r.ndim)) + (0,)
    return np.ascontiguousarray(r.transpose(order))


def kernel(corr, kc, problem, dynamics_logits_table, obs_logits_kc,
           obs_logits_problem, fastbkt_n):
    from concourse.bass_utils import run_bass_kernel_spmd

    corr = np.asarray(corr, dtype=np.float32)
    kc = np.asarray(kc).astype(np.int64)
    problem = np.asarray(problem).astype(np.int64)
    dyn_table = np.asarray(dynamics_logits_table, dtype=np.float32)
    obs_kc = np.asarray(obs_logits_kc, dtype=np.float32)
    obs_prob = np.asarray(obs_logits_problem, dtype=np.float32)

    B, T = corr.shape
    assert B == _B and T == _T, (B, T)

    # host gathers (traffic-neutral input marshaling); slip logit pre-negated
    lls = obs_kc[kc][:, None, :] + obs_prob[problem]       # (B, T, 2)
    lls[:, :, 1] *= -1.0                                   # [lg, -ls]
    dyn = dyn_table[kc]                                    # (B, 3)
    cm8 = (corr * 2.0 - 1.0).astype(np.int8)

    nc = _build()
    in_maps = []
    for core in range(N_CORES):
        in_maps.append({
            "lls2": _shard(lls, core),
            "cm": _shard(cm8, core),
            "dyn": _shard(dyn, core),
        })

    res = run_bass_kernel_spmd(
        nc, in_maps, core_ids=list(range(N_CORES)), **_cached.get("run_kwargs", {})
    )
    _cached["last_results"] = res

    out = np.empty((B, T, 2), np.float32)
    for core in range(N_CORES):
        o = np.asarray(res.results[core]["out"]).astype(np.float32)  # (P,T,2,G)
        rows = o.transpose(3, 0, 1, 2).reshape(P * _G, T, 2)
        out[core * P * _G : (core + 1) * P * _G] = rows
    return out


# revision 12
# speedup vs baseline: 1.7807x; 1.0178x over previous
"""BKT (Bayesian Knowledge Tracing) forward pass on Trainium2, 8 NeuronCores.

The reference's chunked 32-trajectory scan is a 2-state HMM forward pass.
Per (sequence, t):  W_t = diag(o_t) @ Tr  (2x2 per-step matrix),
    alpha' = alpha @ W_t,   p_corr(t) = (alpha . pc_t) / (alpha . 1),
    out = [Ln(1-p), Ln(p)].
Alphas are kept UNNORMALIZED pairs throughout (scale cancels in the
prediction ratio); only occasional power-limited renorms keep fp range.

Device algorithm per core (2048 seqs = 128 partitions x 16 groups, layout
[p, t, s, s', g] with g innermost so every bf16 op hits the DVE 2x mode):
  1. Pool: zpk = lls2 * cm (sign-fold; host pre-negates the slip logit so a
     single sigmoid yields both obs probs).  ACT: op = sigmoid(zpk) -> bf16,
     pc = sigmoid(lls2) -> fp32.
  2. DVE bf16: W = op x Tr; chunk products A_c over K=10 steps (2 fused
     broadcast tensor_tensor per fold, parallel across chunks).
  3. Pool fp32: serial alpha-pair recursion over chunk matrices (2 ops per
     chunk); DVE renorm (approx-recip) every 2 chunks bounds the range.
  4. DVE bf16: within-chunk alpha recovery (1 mult + 1 add per step).
  5. fp32 predictions: qp = alpha*pc, num/den sums, approx-recip, q = 1-p
     (fp32 keeps the cancellation harmless), Ln on ACT, bf16 store (host
     upcasts, which halves the output traffic).

Sharding: pure data-parallel over batch; the tiny per-KC/per-problem tables
are gathered on host (traffic-neutral marshaling), recurrences on device.
"""

import numpy as np

import concourse.bass as bass
import concourse.bacc as bacc
import concourse.tile as tile
import concourse.mybir as mybir

F32 = mybir.dt.float32
BF16 = mybir.dt.bfloat16
I8 = mybir.dt.int8
AF = mybir.ActivationFunctionType
OP = mybir.AluOpType

P = 128
N_CORES = 8


def emit_bkt(nc, G, T, K, SEG, RN=3):
    assert T % SEG == 0 and SEG % K == 0
    NSEG = T // SEG
    CS = SEG // K
    CT = T // K

    lls_d = nc.dram_tensor("lls2", [P, T, 2, G], F32, kind="ExternalInput")
    cm_d = nc.dram_tensor("cm", [P, T, G], I8, kind="ExternalInput")
    dyn_d = nc.dram_tensor("dyn", [P, 3, G], F32, kind="ExternalInput")
    out_d = nc.dram_tensor("out", [P, T, 2, G], BF16, kind="ExternalOutput")

    with tile.TileContext(nc) as tc:
        with (
            tc.tile_pool(name="singles", bufs=1) as singles,
            tc.tile_pool(name="io", bufs=2) as io,
            tc.tile_pool(name="wk2", bufs=2) as wk2,
            tc.tile_pool(name="wk1", bufs=1) as wk1,
        ):
            # ---- per-sequence constants ----
            dyn_t = singles.tile([P, 3, G], F32)
            nc.sync.dma_start(dyn_t[:], dyn_d[:])
            # Tr[s][s'][g]: [[1-l, l], [f, 1-f]]
            Tp = singles.tile([P, 2, 2, G], BF16)
            nc.scalar.activation(Tp[:, 0, 0], dyn_t[:, 0, :], AF.Sigmoid, scale=-1.0)
            nc.scalar.activation(Tp[:, 0, 1], dyn_t[:, 0, :], AF.Sigmoid)
            nc.scalar.activation(Tp[:, 1, 0], dyn_t[:, 1, :], AF.Sigmoid)
            nc.scalar.activation(Tp[:, 1, 1], dyn_t[:, 1, :], AF.Sigmoid, scale=-1.0)
            # chunk-start alpha pairs (unnormalized); start = (1-pI0, pI0)
            starts = singles.tile([P, CT + 1, 2, G], F32)
            nc.scalar.activation(starts[:, 0, 0], dyn_t[:, 2, :], AF.Sigmoid, scale=-1.0)
            nc.scalar.activation(starts[:, 0, 1], dyn_t[:, 2, :], AF.Sigmoid)

            obs = {}
            fin = {}

            def phase_a(seg, nsplit=1):
                s0 = seg * SEG
                lls = io.tile([P, SEG, 2, G], F32, tag="lls")
                cmt = io.tile([P, SEG, G], I8, tag="cm")
                zpk = wk2.tile([P, SEG, 2, G], F32, tag="zpk")
                op_t = wk2.tile([P, SEG, 2, G], BF16, tag="op")
                pc_t = wk2.tile([P, SEG, 2, G], F32, tag="pc")
                bounds = [SEG * h // nsplit for h in range(nsplit + 1)]
                for h in range(nsplit):
                    a, b = bounds[h], bounds[h + 1]
                    nc.sync.dma_start(lls[:, a:b], lls_d[:, s0 + a : s0 + b])
                    nc.sync.dma_start(cmt[:, a:b], cm_d[:, s0 + a : s0 + b])
                    # zpk = lls2 * (2c-1): folds corr into both logits.
                    # Seg 0 runs it on the otherwise-idle DVE (startup ramp).
                    zeng = nc.vector if seg == 0 else nc.gpsimd
                    zeng.tensor_tensor(
                        zpk[:, a:b], lls[:, a:b],
                        cmt[:, a:b].unsqueeze(2).broadcast_to((P, b - a, 2, G)),
                        OP.mult,
                    )
                    # op = [P(obs|L=0), P(obs|L=1)]; pc = [P(c|0), P(c|1)]
                    nc.scalar.activation(op_t[:, a:b], zpk[:, a:b], AF.Sigmoid)
                    nc.scalar.activation(pc_t[:, a:b], lls[:, a:b], AF.Sigmoid)
                obs[seg] = (op_t, pc_t)

            def finalize(seg):
                s0 = seg * SEG
                out_t = fin.pop(seg)
                h = SEG // 2
                nc.sync.dma_start(out_d[:, s0 : s0 + h], out_t[:, :h])
                nc.sync.dma_start(out_d[:, s0 + h : s0 + SEG], out_t[:, h:])

            def phase_b(seg):
                c0 = seg * CS
                op_t, pc_t = obs.pop(seg)

                # W[t][s][s'][g] = op_s(t) * Tr[s][s']  (g innermost: 2x mode)
                Wp = wk2.tile([P, SEG, 2, 2, G], BF16, tag="Wp")
                nc.vector.tensor_tensor(
                    Wp[:],
                    op_t[:].unsqueeze(3).broadcast_to((P, SEG, 2, 2, G)),
                    Tp[:].unsqueeze(1).broadcast_to((P, SEG, 2, 2, G)),
                    OP.mult,
                )
                Wc = Wp[:].rearrange("p (c k) s t g -> p c k s t g", k=K)

                if seg >= 1:
                    finalize(seg - 1)

                # chunk products A_c = W_c0 @ ... @ W_c,K-1  ([c, i, s', g])
                A = wk2.tile([P, CS, 2, 2, G], BF16, tag="A")
                TM = wk2.tile([P, CS, 2, 2, 2, G], BF16, tag="TM")
                nc.vector.tensor_scalar(A[:], Wc[:, :, 0], 1.0, 0.0, OP.mult, OP.add)
                for j in range(1, K):
                    nc.vector.tensor_tensor(
                        TM[:],
                        A[:].unsqueeze(4).broadcast_to((P, CS, 2, 2, 2, G)),
                        Wc[:, :, j].unsqueeze(2).broadcast_to((P, CS, 2, 2, 2, G)),
                        OP.mult,
                    )
                    nc.vector.tensor_tensor(A[:], TM[:, :, :, 0], TM[:, :, :, 1], OP.add)

                # serial alpha-pair recursion over chunks (Pool, fp32)
                sv = wk1.tile([P, 2, 2, G], F32, tag="sv")
                rcp = wk1.tile([P, G], F32, tag="rcp")
                for cl in range(CS):
                    cg = c0 + cl
                    nc.gpsimd.tensor_tensor(
                        sv[:],
                        starts[:, cg].unsqueeze(2).broadcast_to((P, 2, 2, G)),
                        A[:, cl], OP.mult,
                    )
                    nc.gpsimd.tensor_tensor(
                        starts[:, cg + 1], sv[:, 0], sv[:, 1], OP.add
                    )
                    if cg % RN == RN - 1:
                        nc.vector.reciprocal_approx_fast(
                            rcp[:], starts[:, cg + 1, 0]
                        )
                        nc.vector.tensor_tensor(
                            starts[:, cg + 1],
                            starts[:, cg + 1],
                            rcp[:].unsqueeze(1).broadcast_to((P, 2, G)),
                            OP.mult,
                        )

                # within-chunk alpha recovery (bf16): a_j = a_{j-1} @ W_{j-1}
                rec = wk2.tile([P, CS, K, 2, G], BF16, tag="rec")
                nc.gpsimd.tensor_scalar(
                    rec[:, :, 0], starts[:, c0 : c0 + CS], 1.0, 0.0, OP.mult, OP.add
                )
                # RR layout [c, s', i, g]; one mult per target state s'
                RR = wk1.tile([P, CS, 2, 2, G], BF16, tag="RR")
                for j in range(1, K):
                    for sp in range(2):
                        nc.vector.tensor_tensor(
                            RR[:, :, sp],
                            rec[:, :, j - 1],
                            Wc[:, :, j - 1, :, sp, :], OP.mult,
                        )
                    nc.vector.tensor_tensor(
                        rec[:, :, j], RR[:, :, :, 0], RR[:, :, :, 1], OP.add
                    )

                # predictions (fp32): p = (a.pc)/(a.1), q = 1-p
                af = rec[:].rearrange("p c k s g -> p (c k) s g")
                qp0 = wk1.tile([P, SEG, G], F32, tag="qp0")
                qp1 = wk1.tile([P, SEG, G], F32, tag="qp1")
                num = wk1.tile([P, SEG, G], F32, tag="num")
                den = wk1.tile([P, SEG, G], F32, tag="den")
                rdn = wk1.tile([P, SEG, G], F32, tag="rdn")
                pt = wk1.tile([P, SEG, G], F32, tag="pt")
                qt = wk1.tile([P, SEG, G], F32, tag="qt")
                out_t = io.tile([P, SEG, 2, G], BF16, tag="out")
                nsp = 2 if seg == NSEG - 1 else 1
                bounds = [SEG * h // nsp for h in range(nsp + 1)]
                for hh in range(nsp):
                    a, b = bounds[hh], bounds[hh + 1]
                    nc.gpsimd.tensor_tensor(
                        qp0[:, a:b], af[:, a:b, 0], pc_t[:, a:b, 0], OP.mult)
                    nc.vector.tensor_tensor(
                        qp1[:, a:b], af[:, a:b, 1], pc_t[:, a:b, 1], OP.mult)
                    nc.gpsimd.tensor_tensor(
                        den[:, a:b], af[:, a:b, 0], af[:, a:b, 1], OP.add)
                    nc.vector.tensor_tensor(num[:, a:b], qp0[:, a:b], qp1[:, a:b], OP.add)
                    nc.vector.reciprocal_approx_fast(rdn[:, a:b], den[:, a:b])
                    nc.vector.tensor_tensor(pt[:, a:b], num[:, a:b], rdn[:, a:b], OP.mult)
                    nc.vector.tensor_scalar(
                        qt[:, a:b], pt[:, a:b], -1.0, 1.0, OP.mult, OP.add)
                    nc.scalar.activation(out_t[:, a:b, 0], qt[:, a:b], AF.Ln)
                    nc.scalar.activation(out_t[:, a:b, 1], pt[:, a:b], AF.Ln)
                fin[seg] = out_t

            for seg in range(NSEG):
                phase_a(seg, nsplit=(4 if seg == 0 else 1))
                if seg >= 1:
                    phase_b(seg - 1)
            phase_b(NSEG - 1)
            finalize(NSEG - 1)

    return nc


# ------------------------------------------------------------------
# Host-side full-problem wrapper
# ------------------------------------------------------------------

_B, _T, _K, _SEG = 16384, 500, 10, 100
_G = _B // (P * N_CORES)

_cached = {}


def _build():
    if "nc" not in _cached:
        nc = bacc.Bacc(None, target_bir_lowering=False)
        emit_bkt(nc, G=_G, T=_T, K=_K, SEG=_SEG)
        nc.compile()
        _cached["nc"] = nc
    return _cached["nc"]


def _shard(arr, core):
    """(B,...) -> this core's (P, ..., G) permuted view, seq = g*128 + p."""
    rows = arr[core * P * _G : (core + 1) * P * _G]
    r = rows.reshape(_G, P, *arr.shape[1:])
    order = (1,) + tuple(range(2, r.ndim)) + (0,)
    return np.ascontiguousarray(r.transpose(order))


def kernel(corr, kc, problem, dynamics_logits_table, obs_logits_kc,
           obs_logits_problem, fastbkt_n):
    from concourse.bass_utils import run_bass_kernel_spmd

    corr = np.asarray(corr, dtype=np.float32)
    kc = np.asarray(kc).astype(np.int64)
    problem = np.asarray(problem).astype(np.int64)
    dyn_table = np.asarray(dynamics_logits_table, dtype=np.float32)
    obs_kc = np.asarray(obs_logits_kc, dtype=np.float32)
    obs_prob = np.asarray(obs_logits_problem, dtype=np.float32)

    B, T = corr.shape
    assert B == _B and T == _T, (B, T)

    # host gathers (traffic-neutral input marshaling); slip logit pre-negated
    lls = obs_kc[kc][:, None, :] + obs_prob[problem]       # (B, T, 2)
    lls[:, :, 1] *= -1.0                                   # [lg, -ls]
    dyn = dyn_table[kc]                                    # (B, 3)
    cm8 = (corr * 2.0 - 1.0).astype(np.int8)

    nc = _build()
    in_maps = []
    for core in range(N_CORES):
        in_maps.append({
            "lls2": _shard(lls, core),
            "cm": _shard(cm8, core),
            "dyn": _shard(dyn, core),
        })

    res = run_bass_kernel_spmd(
        nc, in_maps, core_ids=list(range(N_CORES)), **_cached.get("run_kwargs", {})
    )
    _cached["last_results"] = res

    out = np.empty((B, T, 2), np.float32)
    for core in range(N_CORES):
        o = np.asarray(res.results[core]["out"]).astype(np.float32)  # (P,T,2,G)
        rows = o.transpose(3, 0, 1, 2).reshape(P * _G, T, 2)
        out[core * P * _G : (core + 1) * P * _G] = rows
    return out
